# revision 42
# baseline (speedup 1.0000x reference)
"""Masked multi-head SDP attention, 8 NeuronCores = (batch, head-half).

B=4, S=2048, D=1024, H=16, DK=64. Core c owns batch c//2 and heads
[(c%2)*8, (c%2)*8+8), processed as 4 head-pair groups. All matmuls bf16.

Cost-model-driven redesign vs the previous version:
- x^T is pre-transposed on the host and loaded as [d, s] directly (no PE
  or XBAR transposes of x on device).
- attn@V runs with queries on the PSUM partition dim: per (j, q-tile,
  head) one matmul [128q x 65] whose 65th column is the ones-product
  softmax denominator. This halves attn@V PE cost vs. replicating the
  denominator across 64 rows (matmul cost = output free size).
- V is projected directly into [s, dk] layout (lhsT = x^T tile, moving =
  Wv), so no V transposes. V's bias is folded into the host-side output
  bias (bv @ Wo is a constant row added with bo).
- cat [q, dk] is DMA-transposed (XBAR) into [dk, s] for the output
  projection; zero PE cost.
- Emission is software-pipelined exactly like before: one chained
  projection generator pumped by attention, output-projection halves
  pushed as fillers into the last group's stream.
"""

import sys

sys.path.insert(0, "/opt/trn_rl_repo")

import collections
import numpy as np
import ml_dtypes

import concourse.bass as bass
import concourse.mybir as mybir
from concourse import bacc
from concourse.masks import make_identity, make_upper_triangular
from concourse.tile import TileContext
from concourse.bass_utils import run_bass_kernel_spmd

B, S, D, H = 4, 2048, 1024, 16
DK = D // H  # 64
NCORES = 8
NG = 4  # head-pair groups per core
KH = 2 * DK  # 128 per group
KC = NG * KH  # 512 projection outputs per core
NT = S // 128  # 16 t-tiles
NI = S // 512  # 4 i-blocks
DC = D // 128  # 8 contraction tiles

F32 = mybir.dt.float32
BF16 = mybir.dt.bfloat16

VW = DK + 1  # 65: [v | ones] per (t-tile, head) in vna
N_WARM = 10  # PE warmup matmuls (tuned against the timeline sim)
LAG = 3  # t-tiles attn@V trails the scores/exp pipeline by
TAIL_SWAP = False  # drain the final i-block 12,13,15,14
CD = 4  # outproj pop delay in flushes
W_EARLY = 0  # warm matmuls interleaved into group-0 st0/st1 projections



def build_nc():
    nc = bacc.Bacc("TRN2", target_bir_lowering=False, debug=False,
                   num_devices=NCORES)
    xt = nc.dram_tensor("xt", [DC, 128, S], BF16, kind="ExternalInput").ap()
    wq = nc.dram_tensor("wq", [NG, DC, 128, KH], BF16,
                        kind="ExternalInput").ap()
    wk = nc.dram_tensor("wk", [NG, DC, 128, KH], BF16,
                        kind="ExternalInput").ap()
    wv = nc.dram_tensor("wv", [NG, DC, 128, KH], BF16,
                        kind="ExternalInput").ap()
    bqk = nc.dram_tensor("bqk", [128, 2 * NG], F32,
                         kind="ExternalInput").ap()
    wo = nc.dram_tensor("wo", [NG, KH, D], BF16, kind="ExternalInput").ap()
    out = nc.dram_tensor("out", [S, D], BF16, kind="ExternalOutput").ap()

    with TileContext(nc) as tc:
        with (
            tc.tile_pool(name="const", bufs=1) as cpool,
            tc.tile_pool(name="seq", bufs=2) as qpool,
            tc.tile_pool(name="vn", bufs=2) as vpool,
            tc.tile_pool(name="attn", bufs=2) as apool,
            tc.tile_pool(name="cats", bufs=2) as cspool,
            tc.tile_pool(name="fin", bufs=4) as fpool,
            tc.tile_pool(name="cat", bufs=1) as catpool,
            tc.tile_pool(name="pacc", bufs=2, space="PSUM") as ps_acc,
            tc.tile_pool(name="psc", bufs=2, space="PSUM") as ps_sc,
            tc.tile_pool(name="pv", bufs=1, space="PSUM") as ps_v,
        ):
            # Load order minimizes time until the first projection group
            # can run: wq, then x^T s-columns 0:512 of every d-tile, then
            # the rest.
            xsb = cpool.tile([128, DC * S], BF16, tag="xsb")
            xts = [xsb[:, dc * S:(dc + 1) * S] for dc in range(DC)]
            # PE warmup: dummy matmuls fill the initial DMA wait so the
            # tensor engine is past its p-state ramp (and never idle) when
            # the first projection lands.
            warm = cpool.tile([128, 512], BF16, tag="warm")
            nc.gpsimd.memset(warm[:], 0.0)
            ident = cpool.tile([128, 128], BF16, tag="ident")
            make_identity(nc, ident[:])
            for w_i in range(N_WARM):
                wps = ps_acc.tile([128, 512], F32, tag="acc",
                                  name=f"warm_{w_i}")
                nc.tensor.matmul(wps[:], warm[:, 0:128], warm[:],
                                 start=True, stop=True)
            # Weights load per-group so group 0 (1.5us of DMA) unblocks the
            # pipeline start, with x^T quarters interleaved so quarter i
            # lands before the group-0 projection of s-block i needs it.
            w_sb = {}
            w_srcs = {"q": wq, "k": wk, "v": wv}
            for nm in ("q", "k", "v"):
                w_sb[nm] = cpool.tile([128, DC * KC], BF16, tag="w" + nm,
                                      name=f"w_{nm}")

            def load_w(nm, g, dc0=0, dc1=DC):
                dst = w_sb[nm][:].rearrange("p (c g k) -> p c g k", c=DC,
                                            g=NG)
                nc.sync.dma_start(
                    out=dst[:, dc0:dc1, g, :],
                    in_=w_srcs[nm][g, dc0:dc1].rearrange("c p k -> p c k"))

            xsb_r = xsb[:].rearrange("p (c s) -> p c s", c=DC)

            def load_xq(q, dc0=0, dc1=DC):
                nc.sync.dma_start(
                    out=xsb_r[:, dc0:dc1, q * 512:(q + 1) * 512],
                    in_=xt[dc0:dc1, :, q * 512:(q + 1) * 512].rearrange(
                        "c p s -> p c s"))

            # interleave group-0 weight and x^T chunks so the first
            # projection matmuls start as early as possible
            load_w("q", 0, 0, 4)
            load_xq(0, 0, 4)
            load_w("q", 0, 4, 8)
            load_xq(0, 4, 8)
            load_w("k", 0)
            load_w("v", 0)
            bqk_sb = cpool.tile([128, 2 * NG], F32, tag="bqk")
            nc.sync.dma_start(out=bqk_sb[:], in_=bqk)
            b_sb = {"q": bqk_sb[:, 0:NG], "k": bqk_sb[:, NG:2 * NG]}
            tri2_sb = cpool.tile([128, 256], BF16, tag="tri")
            make_upper_triangular(nc, tri2_sb[:, 0:128])
            make_upper_triangular(nc, tri2_sb[:, 128:256])
            load_xq(1)
            for nm in ("q", "k", "v"):
                load_w(nm, 1)
            load_xq(2)
            for nm in ("q", "k", "v"):
                load_w(nm, 2)
            load_xq(3)
            for nm in ("q", "k", "v"):
                load_w(nm, 3)
            wo_sb = [cpool.tile([KH, D], BF16, tag=f"wo{g}", name=f"wo_{g}")
                     for g in range(NG)]
            for g in range(NG):
                nc.sync.dma_start(out=wo_sb[g][:], in_=wo[g])

            def prepare(g):
                """Per-group tiles (vna with ones cols, qt, kt)."""
                vna = vpool.tile([128, NT * 2 * VW], BF16, tag="vna",
                                 name=f"vna_{g}")
                vna_r = vna[:].rearrange("p (j h c) -> p j h c", j=NT, h=2)
                nc.gpsimd.memset(vna_r[:, :, :, DK:DK + 1], 1.0)
                qt = qpool.tile([128, S], BF16, tag="qt", name=f"qt_{g}")
                kt = qpool.tile([128, S], BF16, tag="kt", name=f"kt_{g}")
                return dict(g=g, vna=vna, vna_r=vna_r, qt=qt, kt=kt)

            def proj_gen(ctx):
                g = ctx["g"]
                for st in range(NI):
                    sl = slice(st * 512, (st + 1) * 512)
                    if g == 0 and st < 2 and W_EARLY:
                        # fillers for the DMA-paced startup stretch
                        for w_i in range(W_EARLY):
                            wps = ps_acc.tile([128, 512], F32, tag="acc",
                                              name=f"we_{st}_{w_i}")
                            nc.tensor.matmul(wps[:], warm[:, 0:128],
                                             warm[:], start=True, stop=True)
                    for nm in ("q", "k"):
                        acc = ps_acc.tile([128, 512], F32, tag="acc",
                                          name=f"acc_{g}_{st}_{nm}")
                        for dc0 in range(0, DC, 2):
                            for dc in (dc0, dc0 + 1):
                                nc.tensor.matmul(
                                    acc[:],
                                    w_sb[nm][:, dc * KC + g * KH:
                                             dc * KC + (g + 1) * KH],
                                    xts[dc][:, sl], start=(dc == 0),
                                    stop=(dc == DC - 1))
                            yield
                        dst = ctx["qt"] if nm == "q" else ctx["kt"]
                        nc.vector.tensor_scalar_add(
                            dst[:, sl], acc[:], b_sb[nm][:, g:g + 1])
                    # V in [s, dk] layout: lhsT = x^T tile, moving = Wv.
                    accv = ps_acc.tile([128, 512], F32, tag="acc",
                                       name=f"accv_{g}_{st}")
                    for jj in range(4):
                        j = 4 * st + jj
                        reg = accv[:, jj * 128:(jj + 1) * 128]
                        for dc0 in range(0, DC, 2):
                            for dc in (dc0, dc0 + 1):
                                nc.tensor.matmul(
                                    reg,
                                    xts[dc][:, j * 128:(j + 1) * 128],
                                    w_sb["v"][:, dc * KC + g * KH:
                                              dc * KC + (g + 1) * KH],
                                    start=(dc == 0), stop=(dc == DC - 1))
                            yield
                        nc.vector.tensor_copy(
                            ctx["vna_r"][:, j, :, 0:DK],
                            reg.rearrange("p (h c) -> p h c", h=2))

            YPG = 24  # proj_gen yields per st-group

            oneshot = collections.deque()
            gen_box = [None]

            def pull_gen(n=1):
                for _ in range(n):
                    if gen_box[0] is not None:
                        try:
                            next(gen_box[0])
                            continue
                        except StopIteration:
                            gen_box[0] = None
                    if oneshot:
                        oneshot.popleft()()

            def drain_all():
                while oneshot or gen_box[0] is not None:
                    pull_gen(1)

            catts = [None] * NG

            def outproj_half(st, half, tail=False):
                """pw[128,512] = sum_g catt_g[:, st] @ wo_g[:, half]."""
                def emit():
                    ob = obs[st]
                    pw = ps_acc.tile([128, 512], F32, tag="acc",
                                     name=f"pw_{st}_{half}")
                    for g in range(NG):
                        nc.tensor.matmul(
                            pw[:], catts[g][:, st * 128:(st + 1) * 128],
                            wo_sb[g][:, half * 512:(half + 1) * 512],
                            start=(g == 0), stop=(g == NG - 1))
                    dst = ob[:, half * 512:(half + 1) * 512]
                    if half == 0:
                        nc.scalar.copy(dst, pw[:])
                    else:
                        nc.vector.tensor_copy(dst, pw[:])
                    if tail and st >= 14:
                        # split the last tiles' writes so the final DMA
                        # starts as soon as each half lands
                        nc.sync.dma_start(
                            out=out[st * 128:(st + 1) * 128,
                                    half * 512:(half + 1) * 512], in_=dst)
                    elif half == 1:
                        nc.sync.dma_start(
                            out=out[st * 128:(st + 1) * 128, :], in_=ob[:])
                return emit

            obs = {}

            def attention(ctx, pump=None):
                g = ctx["g"]
                qt, kt, vna_r = ctx["qt"], ctx["kt"], ctx["vna_r"]
                catt = catpool.tile([128, S], BF16, tag=f"catt{g}",
                                    name=f"catt_{g}")
                catts[g] = catt
                last = g == NG - 1
                delayed = []

                def tick_delayed():
                    for e in delayed:
                        e[0] -= 1
                    while delayed and delayed[0][0] <= 0:
                        oneshot.append(delayed.pop(0)[1])

                for i in range(NI):
                    if pump is not None:
                        pump(i)
                    nj = 4 * i + 4
                    vps = [ps_v.tile([128, 512], F32, tag=f"v{h}",
                                     name=f"vp_{g}_{i}_{h}")
                           for h in range(2)]
                    cat_i = cspool.tile([128, 512], BF16, tag="cat",
                                        name=f"cat_{g}_{i}")
                    cat_r = cat_i[:].rearrange("p (r h c) -> p r h c",
                                               r=4, h=2)
                    sq0 = i * 512

                    def flush_v(item):
                        """attn@V for one t-tile; on the diagonal tile,
                        q-tile j-4i is complete: normalize + transpose it
                        eagerly so the output projection never waits."""
                        tick_delayed()
                        j, off, at = item
                        at2 = at[:].rearrange("p (h c) -> p h c", h=2)
                        for h in range(2):
                            rhs = vna_r[:, j, h, :]
                            for qq in range(4):
                                qtile = 4 * i + qq
                                if qtile < j:
                                    continue
                                # start only on the first write to the
                                # tile: its pending-zero is bank-wide, so
                                # the sibling regions' first writes replace
                                # rather than accumulate; a second
                                # start=True would wipe their partials.
                                nc.tensor.matmul(
                                    vps[h][:, qq * 128:qq * 128 + VW],
                                    at2[:, h, qq * 128:(qq + 1) * 128],
                                    rhs, start=(j == 0 and qq == 0),
                                    stop=(j == qtile), skip_group_check=True)
                        qq = j - 4 * i
                        if qq < 0:
                            return
                        st = 4 * i + qq
                        for h in range(2):
                            vpr = vps[h][:].rearrange("p (r c) -> p r c",
                                                      r=4)
                            rcp = fpool.tile([128, 1], F32, tag="rcp",
                                             name=f"rcp_{g}_{st}_{h}")
                            nc.vector.reciprocal(
                                rcp[:].rearrange("p (r c) -> p r c", c=1),
                                vpr[:, qq:qq + 1, DK:DK + 1])
                            nc.vector.tensor_scalar_mul(
                                cat_r[:, qq, h, :], vpr[:, qq, 0:DK],
                                rcp[:])
                        if last and i == NI - 1:
                            # final i-block: PE-transpose + copy has a
                            # much shorter latency into the tail output
                            # projection than the XBAR round trip
                            tp = ps_acc.tile([128, 512], F32, tag="acc",
                                             name=f"tp_{st}")
                            tpb = tp[:].bitcast(BF16)
                            for h in range(2):
                                nc.tensor.transpose(
                                    tpb[h * 64:(h + 1) * 64, 0:128],
                                    cat_r[:, qq, h, :], ident[:])
                            nc.scalar.copy(
                                catt[:, st * 128:(st + 1) * 128],
                                tpb[:, 0:128])
                        else:
                            nc.sync.dma_start(
                                out=catt[:, st * 128:(st + 1) * 128],
                                in_=cat_i[:, qq * 128:(qq + 1) * 128],
                                transpose=True)
                        if last:
                            obs[st] = fpool.tile([128, D], BF16, tag="ob",
                                                 name=f"ob_{st}")
                            for half in range(2):
                                delayed.append([CD, outproj_half(
                                    st, half, tail=(i == NI - 1))])

                    pend = []
                    for j in range(nj):
                        q = j - 4 * i
                        off = 128 * q if q >= 0 else 0
                        sp = ps_sc.tile([128, 1024], F32, tag="sc",
                                        name=f"sp_{g}_{i}_{j}")
                        for h in range(2):
                            ks = slice(h * DK, (h + 1) * DK)
                            nc.tensor.matmul(
                                sp[:, h * 512 + off:h * 512 + 512],
                                kt[ks, j * 128:(j + 1) * 128],
                                qt[ks, sq0 + off:sq0 + 512],
                                start=True, stop=True)
                        at = apool.tile([128, 1024], BF16, tag=f"at{j}",
                                        name=f"at_{g}_{i}_{j}")
                        sp2 = sp[:].rearrange("p (r c) -> p r c", r=2)
                        at2 = at[:].rearrange("p (r c) -> p r c", r=2)
                        nc.scalar.activation(
                            at2[:, :, off:512], sp2[:, :, off:512],
                            mybir.ActivationFunctionType.Exp, scale=0.125)
                        if q >= 0:
                            nc.vector.tensor_mul(
                                at2[:, :, off:off + 128],
                                at2[:, :, off:off + 128],
                                tri2_sb[:].rearrange("p (r c) -> p r c", r=2))
                        pend.append((j, off, at))
                        if len(pend) > LAG:
                            flush_v(pend.pop(0))
                        pull_gen(1)
                    if TAIL_SWAP and last and i == NI - 1 and len(pend) == 4:
                        # drain the last block 12,13,15,14 so the final
                        # DMA chain hangs off st14 while st15's overlaps
                        # st14's output projection
                        order = [0, 1, 3, 2]
                        items = list(pend)
                        pend = []
                        for ix in order:
                            flush_v(items[ix])
                            pull_gen(1)
                    while pend:
                        flush_v(pend.pop(0))
                        pull_gen(1)
                for _e in delayed:
                    oneshot.append(_e[1])
                del delayed[:]

            # All projection groups flow through one chained generator.
            progress = [0] * NG
            ctxs = {}

            def chain():
                for gg in range(NG):
                    ctxs[gg] = prepare(gg)
                    for item in proj_gen(ctxs[gg]):
                        progress[gg] += 1
                        yield item

            gen_box[0] = chain()

            def pump(g, i):
                target = (i + 1) * YPG
                while gen_box[0] is not None and progress[g] < target:
                    pull_gen(1)

            for g in range(NG):
                while g not in ctxs and gen_box[0] is not None:
                    pull_gen(1)
                attention(ctxs[g], pump=lambda i, g=g: pump(g, i))
            drain_all()
    nc.finalize()
    return nc


_NC_CACHE = {}


def _get_nc():
    if "nc" not in _NC_CACHE:
        _NC_CACHE["nc"] = build_nc()
    return _NC_CACHE["nc"]


def kernel(x, Wq, bq, Wk, bk, Wv, bv, Wo, bo):
    x_f32 = np.asarray(x, dtype=np.float32)
    Wo_f = np.asarray(Wo, dtype=np.float32)
    # x^T per batch, shared by the two cores of each batch
    xts = [np.ascontiguousarray(x_f32[b].T).reshape(DC, 128, S).astype(
        ml_dtypes.bfloat16) for b in range(B)]
    in_maps = []
    for c in range(NCORES):
        b, half = c // 2, c % 2
        hs = [half * 8 + k for k in range(8)]
        m = {"xt": xts[b]}
        wo_g = np.stack([
            np.concatenate([Wo_f[hs[2 * g] * DK:(hs[2 * g] + 1) * DK],
                            Wo_f[hs[2 * g + 1] * DK:
                                 (hs[2 * g + 1] + 1) * DK]],
                           axis=0)
            for g in range(NG)])
        m["wo"] = np.ascontiguousarray(wo_g.astype(ml_dtypes.bfloat16))
        for nm, W, bb in (("q", Wq, bq), ("k", Wk, bk), ("v", Wv, None)):
            Wg = np.stack([
                np.concatenate([np.asarray(W[hs[2 * g]], np.float32),
                                np.asarray(W[hs[2 * g + 1]], np.float32)],
                               axis=1).reshape(DC, 128, KH)
                for g in range(NG)])  # [NG, DC, 128, KH]
            m["w" + nm] = np.ascontiguousarray(
                Wg.astype(ml_dtypes.bfloat16))
            if bb is not None:
                bc = np.concatenate([np.asarray(bb[h], np.float32)
                                     for h in hs])  # [512]
                m["b" + nm] = np.ascontiguousarray(
                    bc.reshape(NG, 128).T.astype(np.float32))
        m["bqk"] = np.ascontiguousarray(
            np.concatenate([m.pop("bq"), m.pop("bk")], axis=1))
        in_maps.append(m)
    nc = _get_nc()
    res = run_bass_kernel_spmd(nc, in_maps, list(range(NCORES)))
    outp = np.zeros((B, S, D), dtype=np.float32)
    for c in range(NCORES):
        outp[c // 2] += np.asarray(res.results[c]["out"], dtype=np.float32)
    # V bias folded through Wo: out += (bv_flat @ Wo + bo)
    bv_flat = np.asarray(bv, np.float32).reshape(D)
    bo_eff = np.asarray(bo, dtype=np.float32) + bv_flat @ Wo_f
    return outp + bo_eff[None, None, :]
